# revision 1
# baseline (speedup 1.0000x reference)
"""Causal multi-head attention (B=1, S=4096, D=1024, 16 heads) on 8 TRN2
NeuronCores, head-sharded (tensor parallel): 2 heads per core.

Per-core layout strategy:
  - Host pre-transposes x -> x^T [1024, 4096]; per-core weight slices:
    Wq/Wk/Wv columns c*128:(c+1)*128, Wo rows c*128:(c+1)*128.
  - Q^T, K^T computed in [feature, seq] layout (128 rows = 2 heads x 64);
    V computed the same way then PE-transposed to natural [seq, feature]
    with an all-ones column appended per head.
  - Scores computed transposed per 128-wide k-block: S^T[k, q] =
    (K^T slice).T @ Q^T, the two heads row-packed into the PE array
    (each contributes contract dim 64).
  - Softmax without max-subtraction (scores are O(1) gaussian): exp on the
    scalar engine straight out of PSUM; causal masking by trimming matmul
    columns to the causal frontier plus one 128x128 0/1 triangle multiply
    on diagonal blocks.
  - AV: out^T[hd, q] accumulated over k-blocks in PSUM with the ones-column
    producing the softmax denominator as row 64 for free.
  - Normalize: reciprocal of the denominator row, partition-broadcast via a
    K=1 matmul with a ones vector, one elementwise multiply.
  - Output projection: out^T[d_model, s] = Wo_slice.T-free matmul with
    attn^T as the moving operand; per-core partial outputs are summed on
    the host (row-parallel Wo => partial sums).
  - float32r (TF32-like PE path) everywhere: ~4x faster than fp32 matmul,
    measured end-to-end rel err ~3e-4.
"""
import sys
sys.path.insert(0, '/opt/trn_rl_repo')
import numpy as np
import concourse.bacc as bacc
import concourse.tile as tile
from concourse import mybir

F32R = mybir.dt.float32r
F32 = mybir.dt.float32

S = 4096          # sequence length
D = 1024          # d_model
P = 128           # partitions / per-core feature width (2 heads x 64)
NSB = S // 512    # 8 s-blocks of 512
NKC = D // P      # 8 contraction chunks for projections
SCALE = 0.125     # 1/sqrt(64)


def _emit_body(tc, nc, ap):
    all_pools = []

    def pool(**kw):
        p = tc.alloc_tile_pool(**kw)
        all_pools.append(p)
        return p

    consts = pool(name="consts", bufs=1)
    big = pool(name="big", bufs=1)
    xs_pool = pool(name="xs", bufs=2)
    vt_pool = pool(name="vt", bufs=2)
    es_pool = pool(name="es", bufs=3)
    at_pool = pool(name="at", bufs=2)
    nrm_pool = pool(name="nrm", bufs=2)
    po_pool = pool(name="po", bufs=2)
    ps_sc = pool(name="ps_sc", bufs=2, space="PSUM")
    ps_av = pool(name="ps_av", bufs=2, space="PSUM")
    ps_qkv = pool(name="ps_qkv", bufs=2, space="PSUM")

    w_q = consts.tile([P, NKC, P], F32R, tag="w_q")
    w_k = consts.tile([P, NKC, P], F32R, tag="w_k")
    w_v = consts.tile([P, NKC, P], F32R, tag="w_v")
    w_o = consts.tile([P, D], F32R, tag="w_o")
    maskt = consts.tile([P, P], F32R, tag="mask")
    ident = consts.tile([P, P], F32R, tag="ident")
    ones64 = consts.tile([1, 64], F32R, tag="ones64")
    nc.vector.memset(ones64.bitcast(F32), 1.0)
    nc.sync.dma_start(out=ident, in_=ap["ident"])
    nc.sync.dma_start(out=w_q, in_=ap["wq"].rearrange("(c p) m -> p c m", p=P))
    nc.sync.dma_start(out=w_k, in_=ap["wk"].rearrange("(c p) m -> p c m", p=P))
    nc.sync.dma_start(out=w_v, in_=ap["wv"].rearrange("(c p) m -> p c m", p=P))

    def load_late_consts():
        nc.sync.dma_start(out=w_o, in_=ap["wo"])
        nc.sync.dma_start(out=maskt, in_=ap["mask"])

    qt = [big.tile([P, 512], F32R, tag=f"qt{i}", name=f"qt{i}") for i in range(NSB)]
    kt = [big.tile([P, 512], F32R, tag=f"kt{i}", name=f"kt{i}") for i in range(NSB)]
    # V natural per 128-k-block: [v_h0 (64) | 1 | v_h1 (64) | 1]
    vnat = [big.tile([P, 4, 130], F32R, tag=f"vn{i}", name=f"vn{i}") for i in range(NSB)]
    for i in range(NSB):
        nc.vector.memset(vnat[i].bitcast(F32), 1.0)

    xT_r = ap["xT"].rearrange("(c p) s -> p c s", p=P)
    outT_r = ap["outT"].rearrange("(c p) s -> p c s", p=P)

    def emit_qkv(sb):
        xs = xs_pool.tile([P, NKC, 512], F32R, tag="xs")
        for kc in range(NKC):
            nc.sync.dma_start(out=xs[:, kc, :],
                              in_=xT_r[:, kc, sb * 512:(sb + 1) * 512])
        for proj, wt in ((0, w_q), (1, w_k), (2, w_v)):
            ps = ps_qkv.tile([P, 512], F32, tag="qkv")
            for kc in range(NKC):
                nc.tensor.matmul(ps, lhsT=wt[:, kc, :], rhs=xs[:, kc, :],
                                 start=(kc == 0), stop=(kc == NKC - 1))
            if proj == 0:
                nc.vector.tensor_scalar_mul(qt[sb], ps, SCALE)
            elif proj == 1:
                nc.vector.tensor_copy(out=kt[sb], in_=ps)
            else:
                vt = vt_pool.tile([P, 512], F32R, tag="vt")
                nc.vector.tensor_copy(out=vt, in_=ps)
                for t in range(4):
                    pt = ps_qkv.tile([P, P], F32R, tag="qkv")
                    nc.tensor.transpose(pt, vt[:, t * P:(t + 1) * P], ident)
                    nc.vector.tensor_copy(out=vnat[sb][:, t, 0:64], in_=pt[:, 0:64])
                    nc.vector.tensor_copy(out=vnat[sb][:, t, 65:129], in_=pt[:, 64:128])

    def emit_attention(qb, filler):
        nkb = 4 * (qb + 1)
        av0 = ps_av.tile([65, 512], F32, tag="av")
        av1 = ps_av.tile([65, 512], F32, tag="av")
        n_fill = len(filler)
        fill_at = {int((i + 1) * nkb / (n_fill + 1)): i for i in range(n_fill)}
        pending_av = [None]
        for kb in range(nkb):
            sb, t = kb // 4, kb % 4
            j = kb - 4 * qb                 # >= 0 on diagonal blocks
            lo = 128 * j if j > 0 else 0    # first live (unmasked) column
            sc = ps_sc.tile([P, 1024], F32, tag="sc")
            # scores, both heads row-packed (concurrent in the PE array)
            nc.tensor.matmul(sc[:, lo:512],
                             lhsT=kt[sb][0:64, t * P:(t + 1) * P],
                             rhs=qt[qb][0:64, lo:512], start=True, stop=True)
            nc.tensor.matmul(sc[:, 512 + lo:1024],
                             lhsT=kt[sb][64:128, t * P:(t + 1) * P],
                             rhs=qt[qb][64:128, lo:512], start=True, stop=True)
            es = es_pool.tile([P, 1024], F32R, tag="es")
            if lo == 0:
                nc.scalar.activation(out=es, in_=sc,
                                     func=mybir.ActivationFunctionType.Exp)
            else:
                nc.scalar.activation(out=es[:, lo:512], in_=sc[:, lo:512],
                                     func=mybir.ActivationFunctionType.Exp)
                nc.scalar.activation(out=es[:, 512 + lo:1024],
                                     in_=sc[:, 512 + lo:1024],
                                     func=mybir.ActivationFunctionType.Exp)
            if j >= 0:
                # only the [128, 128] triangle at the causal frontier needs masking
                nc.vector.tensor_tensor(out=es[:, lo:lo + 128],
                                        in0=es[:, lo:lo + 128],
                                        in1=maskt,
                                        op=mybir.AluOpType.mult)
                nc.vector.tensor_tensor(out=es[:, 512 + lo:512 + lo + 128],
                                        in0=es[:, 512 + lo:512 + lo + 128],
                                        in1=maskt,
                                        op=mybir.AluOpType.mult)
            first, last = (kb == 0), (kb == nkb - 1)

            # software-pipeline: this block's AV is emitted after the NEXT
            # block's scores so the PE never waits on this block's exp
            def emit_av(sb=sb, t=t, lo=lo, es=es, first=first, last=last):
                nc.tensor.matmul(av0[:, lo:512], lhsT=vnat[sb][:, t, 0:65],
                                 rhs=es[:, lo:512], start=first, stop=last,
                                 skip_group_check=True)
                nc.tensor.matmul(av1[:, lo:512], lhsT=vnat[sb][:, t, 65:130],
                                 rhs=es[:, 512 + lo:1024], start=first,
                                 stop=last, skip_group_check=True)
            if pending_av[0] is not None:
                pending_av[0]()
            pending_av[0] = emit_av
            if kb in fill_at:
                filler[fill_at[kb]]()
        if pending_av[0] is not None:
            pending_av[0]()
        # normalize: recip of denom rows, broadcast via K=1 matmul, multiply
        r0 = nrm_pool.tile([1, 512], F32R, tag="r0")
        r1 = nrm_pool.tile([1, 512], F32R, tag="r1")
        nc.vector.reciprocal(out=r0, in_=av0[64:65, :])
        nc.vector.reciprocal(out=r1, in_=av1[64:65, :])
        bc = nrm_pool.tile([P, 512], F32R, tag="bc")
        for r, lo in ((r0, 0), (r1, 64)):
            bb = ps_qkv.tile([64, 512], F32, tag="qkv")
            nc.tensor.matmul(bb, lhsT=ones64, rhs=r, start=True, stop=True)
            nc.vector.tensor_copy(out=bc[lo:lo + 64, :], in_=bb)
        at = at_pool.tile([P, 512], F32R, tag="at")
        nc.vector.tensor_tensor(out=at[0:64, :], in0=av0[0:64, :],
                                in1=bc[0:64, :], op=mybir.AluOpType.mult)
        nc.vector.tensor_tensor(out=at[64:128, :], in0=av1[0:64, :],
                                in1=bc[64:128, :], op=mybir.AluOpType.mult)

        def emit_proj(qb=qb, at=at):
            po = po_pool.tile([P, NKC, 512], F32R, tag="po")
            for mc in range(NKC):
                pp = ps_qkv.tile([P, 512], F32, tag="qkv")
                nc.tensor.matmul(pp, lhsT=w_o[:, mc * P:(mc + 1) * P], rhs=at,
                                 start=True, stop=True)
                nc.vector.tensor_copy(out=po[:, mc, :], in_=pp)
            nc.sync.dma_start(out=outT_r[:, :, qb * 512:(qb + 1) * 512], in_=po)
        return emit_proj

    # schedule: pipeline QKV(sb+1) and proj(qb-1) into attention(qb)'s slack
    emit_qkv(0)
    load_late_consts()
    pending_proj = None
    for qb in range(NSB):
        filler = []
        if pending_proj is not None:
            filler.append(pending_proj)
        if qb + 1 < NSB:
            filler.append(lambda sb=qb + 1: emit_qkv(sb))
        pending_proj = emit_attention(qb, filler)
    pending_proj()

    for p in reversed(all_pools):
        p.release()


def build(k_repeat=1):
    nc = bacc.Bacc("TRN2", target_bir_lowering=False, debug=False,
                   enable_asserts=False)
    ap = {}
    ap["xT"] = nc.dram_tensor("xT", [D, S], F32R, kind="ExternalInput").ap()
    ap["wq"] = nc.dram_tensor("wq", [D, P], F32R, kind="ExternalInput").ap()
    ap["wk"] = nc.dram_tensor("wk", [D, P], F32R, kind="ExternalInput").ap()
    ap["wv"] = nc.dram_tensor("wv", [D, P], F32R, kind="ExternalInput").ap()
    ap["wo"] = nc.dram_tensor("wo", [P, D], F32R, kind="ExternalInput").ap()
    ap["mask"] = nc.dram_tensor("mask", [P, P], F32R, kind="ExternalInput").ap()
    ap["ident"] = nc.dram_tensor("ident", [P, P], F32R, kind="ExternalInput").ap()
    ap["outT"] = nc.dram_tensor("outT", [D, S], F32R, kind="ExternalOutput").ap()
    with tile.TileContext(nc) as tc, \
         nc.allow_low_precision(reason="float32r PE path; accumulation stays fp32"):
        if k_repeat == 1:
            _emit_body(tc, nc, ap)
        else:
            # hint_engines: the ~960-instruction PE stream spans ~4 IRAM
            # blocks; prefetch hints keep the back-edge from paying an
            # instruction-fetch DMA per iteration (timing builds only)
            with tc.For_i(0, k_repeat, 1,
                          hint_engines=(mybir.EngineType.PE,
                                        mybir.EngineType.Activation,
                                        mybir.EngineType.DVE,
                                        mybir.EngineType.SP)):
                _emit_body(tc, nc, ap)
    nc.compile()
    return nc


def make_in_maps(x, Wq, Wk, Wv, Wo):
    """x [1,S,D] fp32 -> list of 8 per-core input dicts."""
    xT = np.ascontiguousarray(np.asarray(x, dtype=np.float32)[0].T)
    ki = np.arange(P)[:, None]
    qi = np.arange(P)[None, :]
    mask = (qi >= ki).astype(np.float32)       # causal triangle, [128, 128]
    ident = np.eye(P, dtype=np.float32)
    in_maps = []
    for c in range(8):
        cs = slice(c * P, (c + 1) * P)
        in_maps.append({
            "xT": xT,
            "wq": np.ascontiguousarray(np.asarray(Wq, np.float32)[:, cs]),
            "wk": np.ascontiguousarray(np.asarray(Wk, np.float32)[:, cs]),
            "wv": np.ascontiguousarray(np.asarray(Wv, np.float32)[:, cs]),
            "wo": np.ascontiguousarray(np.asarray(Wo, np.float32)[cs, :]),
            "mask": mask,
            "ident": ident,
        })
    return in_maps


def combine(results):
    """Sum 8 partial outT [D, S] tensors and restore [1, S, D] fp32."""
    acc = np.zeros((D, S), dtype=np.float32)
    for r in results:
        acc += np.asarray(r["outT"])
    return np.ascontiguousarray(acc.T)[None, :, :].astype(np.float32)


_NC_CACHE = {}


def kernel(x, Wq, Wk, Wv, Wo):
    from concourse import bass_utils
    if "nc" not in _NC_CACHE:
        _NC_CACHE["nc"] = build(k_repeat=1)
    nc = _NC_CACHE["nc"]
    in_maps = make_in_maps(x, Wq, Wk, Wv, Wo)
    res = bass_utils.run_bass_kernel_spmd(nc, in_maps, core_ids=list(range(8)))
    return combine(res.results)



# revision 7
# speedup vs baseline: 1.3640x; 1.3640x over previous
"""Causal multi-head attention (B=1, S=4096, D=1024, 16 heads) on 8 TRN2
NeuronCores, head-sharded (tensor parallel): 2 heads per core.

v2 design (vs the 334us fp32r baseline):
  - bf16 data path everywhere (PSUM accumulation stays fp32): halves DMA
    and SBUF traffic, removes the fp32r narrow-matmul 4x penalty, and
    enables 2x/4x DVE modes where operands are all-SBUF.
  - Scores computed transposed per 128-wide k-block: S^T[k, q], the two
    heads sharing one [128, 2, 512] PSUM tile; ONE fused exp per k-block
    ([128, 2, 512-lo] strided AP) on the scalar engine.
  - Causal masking with zero vector-engine work: a -60 bias is added to
    the upper triangle of diagonal blocks BY THE PE ITSELF (one extra
    128-wide matmul per head accumulating biasI^T @ stepU, start=False),
    so nothing sits between exp and AV on the critical path.
  - AV accumulated over k-blocks in PSUM with a ones-column per head
    producing the softmax denominator for free (row 64).
  - Normalization: av evacuated to SBUF (frees PSUM banks fast),
    reciprocal via the single-instruction RECIPROCAL_APPROX_FAST custom
    DVE op (~5x faster than iterative divide), partition-broadcast of
    both heads' 1/denom rows with ONE K=2 selector matmul, two
    tensor_tensor mults -> at.
  - V transposed to natural [seq, feat] layout by the DMA crossbar
    (dma_start transpose=True, bf16) instead of PE transposes.
  - Fine-grained software pipelining: QKV(sb+1), normalize(qb-1) and
    output-projection(qb-1) work is chopped into ~30 small closures per
    q-block and drained 1-3 per k-block into the PE idle slots while the
    scalar engine (exp) is the per-block rate limiter. x-tile DMA is
    prefetched one q-block ahead so QKV matmuls never wait on HBM.
  - PSUM budget exactly 8 banks: scores 2x[128,2,512] (4) + av pair (2)
    + shared work pool (2, strictly ring-ordered).
"""
import sys
sys.path.insert(0, '/opt/trn_rl_repo')
import numpy as np
import concourse.bacc as bacc
import concourse.tile as tile
from concourse import mybir

BF = mybir.dt.bfloat16
F32R = mybir.dt.float32r
F32 = mybir.dt.float32

S = 4096          # sequence length
D = 1024          # d_model
P = 128           # partitions / per-core feature width (2 heads x 64)
NSB = S // 512    # 8 s-blocks of 512
NKC = D // P      # 8 contraction chunks for projections
SCALE = 0.125     # 1/sqrt(64), folded into Wq on the host
NEG = -60.0       # causal bias (exp(-60+smax) ~ 0)


def _emit_body(tc, nc, ap):
    all_pools = []
    _ctr = [0]

    def nm():
        _ctr[0] += 1
        return _ctr[0]

    def pool(**kw):
        p = tc.alloc_tile_pool(**kw)
        all_pools.append(p)
        return p

    consts = pool(name="consts", bufs=1)
    big = pool(name="big", bufs=1)
    xs_pool = pool(name="xs", bufs=2)
    vt_pool = pool(name="vt", bufs=2)
    es_pool = pool(name="es", bufs=3)
    at_pool = pool(name="at", bufs=2)
    nrm_pool = pool(name="nrm", bufs=2)
    po_pool = pool(name="po", bufs=2)
    ps_sc = pool(name="ps_sc", bufs=2, space="PSUM")
    ps_av = pool(name="ps_av", bufs=2, space="PSUM")
    ps_w = pool(name="ps_w", bufs=2, space="PSUM")

    w_q = consts.tile([P, NKC, P], BF, tag="w_q")
    w_k = consts.tile([P, NKC, P], BF, tag="w_k")
    w_v = consts.tile([P, NKC, P], BF, tag="w_v")
    w_o = consts.tile([P, D], BF, tag="w_o")
    stepU = consts.tile([P, P], BF, tag="stepU")    # U[k,q] = 1 if q < k
    identM = consts.tile([P, P], BF, tag="identM")
    biasI = consts.tile([P, P], BF, tag="biasI")    # NEG * I
    ones64 = consts.tile([1, 64], BF, tag="ones64")
    nc.vector.memset(ones64, 1.0)
    nc.sync.dma_start(out=w_q, in_=ap["wq"].rearrange("(c p) m -> p c m", p=P))
    nc.sync.dma_start(out=w_k, in_=ap["wk"].rearrange("(c p) m -> p c m", p=P))
    nc.sync.dma_start(out=w_v, in_=ap["wv"].rearrange("(c p) m -> p c m", p=P))

    def load_late_consts():
        nc.sync.dma_start(out=w_o, in_=ap["wo"])
        nc.sync.dma_start(out=stepU, in_=ap["mask"])
        nc.sync.dma_start(out=biasI, in_=ap["biasI"])
        nc.sync.dma_start(out=identM, in_=ap["identM"])

    qt = [big.tile([P, 512], BF, tag=f"qt{i}", name=f"qt{i}") for i in range(NSB)]
    kt = [big.tile([P, 512], BF, tag=f"kt{i}", name=f"kt{i}") for i in range(NSB)]
    # V natural layout per 128-k-block: [P, t, head, 65] with col 64 = ones
    vnat = [big.tile([P, 4, 2, 65], BF, tag=f"vn{i}", name=f"vn{i}")
            for i in range(NSB)]
    for i in range(NSB):
        nc.vector.memset(vnat[i][:, :, :, 64:65], 1.0)

    xT_r = ap["xT"].rearrange("(c p) s -> p c s", p=P)
    outT_r = ap["outT"].rearrange("(c p) s -> p c s", p=P)

    xs_tiles = {}

    def prefetch_x(sb):
        xs = xs_pool.tile([P, NKC, 512], BF, tag="xs")
        for kc in range(NKC):
            nc.sync.dma_start(out=xs[:, kc, :],
                              in_=xT_r[:, kc, sb * 512:(sb + 1) * 512])
        xs_tiles[sb] = xs

    def queue_qkv(sb, units):
        """~19 small closures: q-chain, k-chain, v-chain + dma-transposes."""
        xs = xs_tiles[sb]

        def chain(wt, out_cb):
            cell = {}

            def mk_mm(i, wt=wt, cell=cell):
                def run():
                    if i == 0:
                        cell["ps"] = ps_w.tile([P, 512], F32, tag="w", name=f"psw_{nm()}")
                    for kc in (2 * i, 2 * i + 1):
                        nc.tensor.matmul(cell["ps"], lhsT=wt[:, kc, :],
                                         rhs=xs[:, kc, :], start=(kc == 0),
                                         stop=(kc == NKC - 1))
                return run
            for i in range(4):
                units.append(mk_mm(i))
            units.append(lambda: out_cb(cell["ps"]))

        chain(w_q, lambda ps: nc.vector.tensor_copy(out=qt[sb], in_=ps))
        chain(w_k, lambda ps: nc.vector.tensor_copy(out=kt[sb], in_=ps))
        vcell = {}

        def vt_copy(ps):
            vcell["vt"] = vt_pool.tile([P, 512], BF, tag="vt", name=f"vt_{nm()}")
            nc.vector.tensor_copy(out=vcell["vt"], in_=ps)
        chain(w_v, vt_copy)
        for t in range(4):
            def mk_tr(t=t):
                def run():
                    pt = ps_w.tile([P, 2, 64], BF, tag="w", name=f"psw_{nm()}")
                    nc.tensor.transpose(pt, vcell["vt"][:, t * P:(t + 1) * P],
                                        identM)
                    nc.vector.tensor_copy(out=vnat[sb][:, t, :, 0:64], in_=pt)
                return run
            units.append(mk_tr())

    def queue_norm(av0, av1, at_cell, units):
        """Deferred normalize tail: head-broadcast matmul + 2 mults."""
        avs = nrm_pool.tile([64, 2, 512], F32, tag="avs")
        den = nrm_pool.tile([1, 2, 512], F32, tag="den")
        r2 = nrm_pool.tile([1, 2, 512], F32, tag="r2")
        # inline: evacuate av (frees the PSUM banks fast) + reciprocal.
        # Denominator rows land on partition 0: the custom-DVE fast recip
        # mis-addresses when in/out partition offsets differ on HW.
        nc.vector.tensor_copy(out=den[:, 0, :], in_=av0[64:65, :])
        nc.vector.tensor_copy(out=den[:, 1, :], in_=av1[64:65, :])
        nc.vector.tensor_copy(out=avs[:, 0, :], in_=av0[0:64, :])
        nc.vector.tensor_copy(out=avs[:, 1, :], in_=av1[0:64, :])
        nc.vector.reciprocal_approx_fast(out=r2, in_=den)
        r2b = nrm_pool.tile([1, 2, 512], BF, tag="r2b")
        nc.vector.tensor_copy(out=r2b, in_=r2)
        bcell = {}

        def bb_mm():
            bcell["bb"] = ps_w.tile([P, 512], F32, tag="w", name=f"psw_{nm()}")
            nc.tensor.matmul(bcell["bb"][0:64, :], lhsT=ones64, rhs=r2b[:, 0, :],
                             start=True, stop=True, skip_group_check=True)
            nc.tensor.matmul(bcell["bb"][64:128, :], lhsT=ones64, rhs=r2b[:, 1, :],
                             start=True, stop=True, skip_group_check=True)

        def at_mults():
            at = at_pool.tile([P, 512], BF, tag="at")
            nc.vector.tensor_tensor(out=at[0:64, :], in0=avs[0:64, 0, :],
                                    in1=bcell["bb"][0:64, :],
                                    op=mybir.AluOpType.mult)
            nc.vector.tensor_tensor(out=at[64:128, :], in0=avs[0:64, 1, :],
                                    in1=bcell["bb"][64:128, :],
                                    op=mybir.AluOpType.mult)
            at_cell["at"] = at
        units.append(bb_mm)
        units.append(at_mults)

    def queue_proj(qb, at_cell, units):
        po = po_pool.tile([P, NKC, 512], BF, tag="po")

        def mk(mc):
            def run():
                pp = ps_w.tile([P, 512], F32, tag="w", name=f"psw_{nm()}")
                nc.tensor.matmul(pp, lhsT=w_o[:, mc * P:(mc + 1) * P],
                                 rhs=at_cell["at"], start=True, stop=True)
                if mc % 4 == 3:
                    nc.scalar.copy(out=po[:, mc, :], in_=pp)
                else:
                    nc.vector.tensor_copy(out=po[:, mc, :], in_=pp)
            return run
        for mc in range(NKC):
            units.append(mk(mc))
        units.append(lambda: nc.sync.dma_start(
            out=outT_r[:, :, qb * 512:(qb + 1) * 512], in_=po))

    def emit_attention(qb, units):
        nkb = 4 * (qb + 1)
        av0 = ps_av.tile([65, 512], F32, tag="av")
        av1 = ps_av.tile([65, 512], F32, tag="av")
        pending_av = [None]
        drained = [0]

        def drain(n):
            for _ in range(n):
                if drained[0] < len(units):
                    units[drained[0]]()
                    drained[0] += 1
        for kb in range(nkb):
            sb, t = kb // 4, kb % 4
            j = kb - 4 * qb                 # >= 0 on diagonal blocks
            lo = 128 * j if j > 0 else 0    # first live (unmasked) column
            sc = ps_sc.tile([P, 2, 512], F32, tag="sc")
            nc.tensor.matmul(sc[:, 0, lo:512],
                             lhsT=kt[sb][0:64, t * P:(t + 1) * P],
                             rhs=qt[qb][0:64, lo:512], start=True,
                             stop=(j < 0), skip_group_check=True)
            nc.tensor.matmul(sc[:, 1, lo:512],
                             lhsT=kt[sb][64:128, t * P:(t + 1) * P],
                             rhs=qt[qb][64:128, lo:512], start=True,
                             stop=(j < 0), skip_group_check=True)
            if j >= 0:
                # causal bias on the diagonal square, by the PE itself
                nc.tensor.matmul(sc[:, 0, lo:lo + 128], lhsT=biasI, rhs=stepU,
                                 start=False, stop=True, skip_group_check=True)
                nc.tensor.matmul(sc[:, 1, lo:lo + 128], lhsT=biasI, rhs=stepU,
                                 start=False, stop=True, skip_group_check=True)
            es = es_pool.tile([P, 2, 512], BF, tag="es")
            nc.scalar.activation(out=es[:, :, lo:512], in_=sc[:, :, lo:512],
                                 func=mybir.ActivationFunctionType.Exp)
            # pace the filler units evenly across the block loop
            want = (len(units) * (kb + 1) + nkb - 1) // nkb
            drain(min(want - drained[0], 3))
            first, last = (kb == 0), (kb == nkb - 1)

            def emit_av(sb=sb, t=t, lo=lo, es=es, first=first, last=last):
                nc.tensor.matmul(av0[:, lo:512], lhsT=vnat[sb][:, t, 0, :],
                                 rhs=es[:, 0, lo:512], start=first, stop=last,
                                 skip_group_check=True)
                nc.tensor.matmul(av1[:, lo:512], lhsT=vnat[sb][:, t, 1, :],
                                 rhs=es[:, 1, lo:512], start=first, stop=last,
                                 skip_group_check=True)
            if pending_av[0] is not None:
                pending_av[0]()
            pending_av[0] = emit_av
        pending_av[0]()
        drain(len(units))
        return av0, av1

    # ---- schedule ----------------------------------------------------------
    prefetch_x(0)
    load_late_consts()
    prologue = []
    queue_qkv(0, prologue)
    for u in prologue:
        u()
    prefetch_x(1)

    at_cells = {}
    norm_args = {}
    for qb in range(NSB):
        units = []
        if qb + 2 < NSB:
            prefetch_x(qb + 2)
        if qb + 1 < NSB:
            queue_qkv(qb + 1, units)
        if qb >= 1:
            # normalize tail of qb-1 (bb matmul + at mults), then its proj
            at_cells[qb - 1] = {}
            queue_norm(*norm_args[qb - 1], at_cells[qb - 1], units)
            queue_proj(qb - 1, at_cells[qb - 1], units)
        norm_args[qb] = emit_attention(qb, units)

    units = []
    at_cells[NSB - 1] = {}
    queue_norm(*norm_args[NSB - 1], at_cells[NSB - 1], units)
    queue_proj(NSB - 1, at_cells[NSB - 1], units)
    for u in units:
        u()

    for p in reversed(all_pools):
        p.release()


def build(k_repeat=1):
    nc = bacc.Bacc("TRN2", target_bir_lowering=False, debug=False,
                   enable_asserts=False)
    ap = {}
    ap["xT"] = nc.dram_tensor("xT", [D, S], BF, kind="ExternalInput").ap()
    ap["wq"] = nc.dram_tensor("wq", [D, P], BF, kind="ExternalInput").ap()
    ap["wk"] = nc.dram_tensor("wk", [D, P], BF, kind="ExternalInput").ap()
    ap["wv"] = nc.dram_tensor("wv", [D, P], BF, kind="ExternalInput").ap()
    ap["wo"] = nc.dram_tensor("wo", [P, D], BF, kind="ExternalInput").ap()
    ap["mask"] = nc.dram_tensor("mask", [P, P], BF, kind="ExternalInput").ap()
    ap["biasI"] = nc.dram_tensor("biasI", [P, P], BF, kind="ExternalInput").ap()
    ap["identM"] = nc.dram_tensor("identM", [P, P], BF, kind="ExternalInput").ap()
    ap["outT"] = nc.dram_tensor("outT", [D, S], BF, kind="ExternalOutput").ap()
    with tile.TileContext(nc) as tc, \
         nc.allow_low_precision(reason="bf16 data path; accumulation stays fp32"):
        if k_repeat == 1:
            _emit_body(tc, nc, ap)
        else:
            with tc.For_i(0, k_repeat, 1,
                          hint_engines=(mybir.EngineType.PE,
                                        mybir.EngineType.Activation,
                                        mybir.EngineType.DVE,
                                        mybir.EngineType.SP)):
                _emit_body(tc, nc, ap)
    nc.compile()
    return nc


def _bf16(a):
    import ml_dtypes
    return np.asarray(a, dtype=np.float32).astype(ml_dtypes.bfloat16)


def make_in_maps(x, Wq, Wk, Wv, Wo):
    """x [1,S,D] fp32 -> list of 8 per-core input dicts (bf16)."""
    xT = np.ascontiguousarray(np.asarray(x, dtype=np.float32)[0].T)
    xT = _bf16(xT)
    ki = np.arange(P)[:, None]
    qi = np.arange(P)[None, :]
    stepU = _bf16((qi < ki).astype(np.float32))   # 1 where masked (q < k)
    biasI = _bf16(NEG * np.eye(P, dtype=np.float32))
    Wq = np.asarray(Wq, np.float32) * SCALE
    in_maps = []
    for c in range(8):
        cs = slice(c * P, (c + 1) * P)
        in_maps.append({
            "xT": xT,
            "wq": _bf16(Wq[:, cs]),
            "wk": _bf16(np.asarray(Wk, np.float32)[:, cs]),
            "wv": _bf16(np.asarray(Wv, np.float32)[:, cs]),
            "wo": _bf16(np.asarray(Wo, np.float32)[cs, :]),
            "mask": stepU,
            "biasI": biasI,
            "identM": _bf16(np.eye(P, dtype=np.float32)),
        })
    return in_maps


def combine(results):
    """Sum 8 partial outT [D, S] bf16 tensors and restore [1, S, D] fp32."""
    acc = np.zeros((D, S), dtype=np.float32)
    for r in results:
        acc += np.asarray(r["outT"]).astype(np.float32)
    return np.ascontiguousarray(acc.T)[None, :, :].astype(np.float32)


_NC_CACHE = {}


def kernel(x, Wq, Wk, Wv, Wo):
    from concourse import bass_utils
    if "nc" not in _NC_CACHE:
        _NC_CACHE["nc"] = build(k_repeat=1)
    nc = _NC_CACHE["nc"]
    in_maps = make_in_maps(x, Wq, Wk, Wv, Wo)
    res = bass_utils.run_bass_kernel_spmd(nc, in_maps, core_ids=list(range(8)))
    return combine(res.results)
